# revision 1
# baseline (speedup 1.0000x reference)
"""Multi-head attention (B=8, S=2048, D=512, H=8, DH=64) on 8 TRN2 NeuronCores.

Strategy: data-parallel over the batch dim — core b computes batch element b
end-to-end (no collectives). Per core, everything is kept transposed
("feature on partitions") so that softmax reductions land on the TensorE
contraction axis:

  1. QKV projection with head-interleaved, pre-transposed weights gives
     Q^T, K^T laid out (64h+c, s) and V laid out (s, 64h+c).
  2. Scores are computed transposed, S^T[j, i] = sum_c K^T[c,j] Q^T[c,i],
     as K=64 matmuls row-packed two-at-a-time into disjoint PE row groups
     (partitions 0:64 / 64:128 via lo/hi replicas of Q^T/K^T).
  3. exp(scale * S^T) runs on ScalarE straight out of PSUM into bf16 SBUF.
     ScalarE is the kernel's bottleneck engine; PV matmuls trail the
     score matmuls by one j-chunk so the PE FIFO never stalls the
     exp stream.
  4. O^T[c, i] = sum_j Vaug[j, c] E^T[j, i] with Vaug = [V | ones]: M=65
     matmuls whose 65th row accumulates the softmax denominator for free.
  5. Normalization is decoupled from the PE pipeline: O_un is copied out
     of PSUM (freeing the accumulation banks), the denominator row is
     reshaped to (128,16) for a cheap reciprocal, round-trips through
     DRAM for a partition-broadcast, and a VectorE multiply writes the
     normalized O^T. The output projection (+bias) then produces out^T
     which the host transposes back.
"""

import numpy as np
import ml_dtypes

B, S, D = 8, 2048, 512
H, DH = 8, 64
INNER = H * DH
SCALE = DH ** -0.5

N_CORES = 8
NDT = D // 128   # 4 contraction tiles
NSC = S // 128   # 16 s-chunks (j-chunks)
NST = S // 512   # 4 s-tiles


def _build_kernel():
    import concourse.bass as bass
    import concourse.mybir as mybir
    import concourse.tile as tile
    from concourse import bacc

    bf16 = mybir.dt.bfloat16
    f32 = mybir.dt.float32
    Exp = mybir.ActivationFunctionType.Exp

    nc = bacc.Bacc()

    xT = nc.declare_dram_parameter("xT", [D, S], bf16, isOutput=False)
    wq = nc.declare_dram_parameter("wq", [D, INNER], bf16, isOutput=False)
    wk = nc.declare_dram_parameter("wk", [D, INNER], bf16, isOutput=False)
    wv = nc.declare_dram_parameter("wv", [D, INNER], bf16, isOutput=False)
    wo = nc.declare_dram_parameter("wo", [INNER, D], bf16, isOutput=False)
    bo = nc.declare_dram_parameter("bo", [NDT, 128, 1], f32, isOutput=False)
    out = nc.declare_dram_parameter("out", [D, S], f32, isOutput=True)
    den_dram = nc.dram_tensor("den_scratch", [H, S], f32)

    with tile.TileContext(nc) as tc:
        with (
            tc.tile_pool(name="weights", bufs=1) as wpool,
            tc.tile_pool(name="acts", bufs=1) as apool,
            tc.tile_pool(name="et", bufs=3) as epool,
            tc.tile_pool(name="small", bufs=2) as spool,
            tc.tile_pool(name="ostage", bufs=2) as opool,
            tc.tile_pool(name="psA", bufs=2, space="PSUM") as psA,
            tc.tile_pool(name="psV", bufs=1, space="PSUM") as psV,
        ):
            # ---- load inputs (x and q/k weights first: they gate head 0) ----
            xT_s = [[wpool.tile([128, S // 2], bf16, name=f"xT{d}_{hf}",
                              tag=f"xT{d}_{hf}") for hf in range(2)]
                    for d in range(NDT)]
            wq_s = [wpool.tile([128, INNER], bf16, name=f"wq{d}", tag=f"wq{d}")
                    for d in range(NDT)]
            wk_s = [wpool.tile([128, INNER], bf16, name=f"wk{d}", tag=f"wk{d}")
                    for d in range(NDT)]
            wv_s = [wpool.tile([128, INNER], bf16, name=f"wv{d}", tag=f"wv{d}")
                    for d in range(NDT)]
            wo_s = [wpool.tile([128, D], bf16, name=f"wo{d}", tag=f"wo{d}")
                    for d in range(NDT)]
            bo_s = [wpool.tile([128, 1], f32, name=f"bo{d}", tag=f"bo{d}")
                    for d in range(NDT)]
            for d in range(NDT):
                sl = slice(d * 128, (d + 1) * 128)
                nc.sync.dma_start(out=xT_s[d][0][:], in_=xT[sl, 0:S // 2])
                nc.sync.dma_start(out=wq_s[d][:], in_=wq[sl, :])
                nc.sync.dma_start(out=wk_s[d][:], in_=wk[sl, :])
            for d in range(NDT):
                sl = slice(d * 128, (d + 1) * 128)
                nc.scalar.dma_start(out=xT_s[d][1][:], in_=xT[sl, S // 2:])
            for d in range(NDT):
                sl = slice(d * 128, (d + 1) * 128)
                nc.scalar.dma_start(out=wv_s[d][:], in_=wv[sl, :])
                nc.scalar.dma_start(out=wo_s[d][:], in_=wo[sl, :])
                nc.scalar.dma_start(out=bo_s[d][:], in_=bo[d, :, :])

            # ---- QKV projection ----
            qt_lo = [apool.tile([128, S], bf16, name=f"qlo{t}", tag=f"qlo{t}")
                     for t in range(NDT)]
            kt_lo = [apool.tile([128, S], bf16, name=f"klo{t}", tag=f"klo{t}")
                     for t in range(NDT)]
            qt_hi = [apool.tile([128, S], bf16, name=f"qhi{t}", tag=f"qhi{t}")
                     for t in range(NDT)]
            kt_hi = [apool.tile([128, S], bf16, name=f"khi{t}", tag=f"khi{t}")
                     for t in range(NDT)]

            # PE warm-up: junk matmuls during the input-DMA window keep the
            # HAM activity monitor busy so real matmuls start at 2.4 GHz.
            junk_sb = wpool.tile([128, 512], bf16, name="junk", tag="junk")
            nc.vector.memset(junk_sb[:, :], 0.0)
            junk_ps = psV.tile([128, 4 * 512], f32, name="junkps", tag="pv")
            for k in range(16):
                nc.tensor.matmul(
                    junk_ps[:, (k % 4) * 512:(k % 4 + 1) * 512],
                    lhsT=junk_sb[:, 0:128],
                    rhs=junk_sb[:, :],
                )

            def qk_chunk(w_s, dst, ch):
                for half in range(2):  # s in 1024-halves
                    pa = psA.tile([128, 1024], f32, name="pa", tag="pa")
                    for d in range(NDT):
                        for nn in range(2):
                            s0 = nn * 512
                            nc.tensor.matmul(
                                pa[:, nn * 512:(nn + 1) * 512],
                                lhsT=w_s[d][:, ch * 128:(ch + 1) * 128],
                                rhs=xT_s[d][half][:, s0:s0 + 512],
                                start=(d == 0),
                                stop=(d == NDT - 1),
                            )
                    nc.vector.tensor_copy(
                        dst[ch][:, half * 1024:(half + 1) * 1024], pa[:, :])

            def swap_halves(t):
                for (lo, hi) in ((qt_lo, qt_hi), (kt_lo, kt_hi)):
                    nc.sync.dma_start(out=hi[t][64:128, :], in_=lo[t][0:64, :])
                    nc.sync.dma_start(out=hi[t][0:64, :], in_=lo[t][64:128, :])

            # V first (its psV use and VectorE copies must clear before head
            # 0's PV accumulation), then the remaining Q/K chunks.
            qk_chunk(wq_s, qt_lo, 0)
            qk_chunk(wk_s, kt_lo, 0)
            swap_halves(0)

            v_aug = [apool.tile([128, H * (DH + 1)], bf16, name=f"va{m}",
                                tag=f"va{m}") for m in range(NSC)]

            def v_round(r):
                pvt = psV.tile([128, 4 * 512], f32, name="pvt", tag="pv")
                for k in range(4):
                    m = 4 * r + k
                    for d in range(NDT):
                        mh, mo = divmod(m, 8)
                        nc.tensor.matmul(
                            pvt[:, k * 512:(k + 1) * 512],
                            lhsT=xT_s[d][mh][:, mo * 128:(mo + 1) * 128],
                            rhs=wv_s[d][:, :],
                            start=(d == 0),
                            stop=(d == NDT - 1),
                        )
                for k in range(4):
                    m = 4 * r + k
                    va = v_aug[m].rearrange("p (h t) -> p h t", t=DH + 1)
                    nc.vector.tensor_copy(
                        va[:, :, 0:DH],
                        pvt[:, k * 512:(k + 1) * 512].rearrange(
                            "p (h t) -> p h t", t=DH),
                    )
                    nc.vector.memset(va[:, :, DH:DH + 1], 1.0)

            for r in range(NSC // 4):
                v_round(r)

            for ch in range(1, NDT):
                qk_chunk(wq_s, qt_lo, ch)
                qk_chunk(wk_s, kt_lo, ch)
                swap_halves(ch)

            # ---- attention, head by head ----
            # PE order per j-chunk: scores(jc) then PV(jc-1), so the PE FIFO
            # never waits on exp(jc) before issuing scores(jc+1).
            ot = [apool.tile([128, S], bf16, name=f"ot{t}", tag=f"ot{t}")
                  for t in range(NDT)]
            for h in range(H):
                t, p = h // 2, h % 2
                lo_sl = slice(64 * p, 64 * p + 64)
                hi_sl = slice(64 * (1 - p), 64 * (1 - p) + 64)
                pv = psV.tile([128, 4 * 512], f32, name="pvh", tag="pv")
                ets = {}

                def pv_mms(jc):
                    for it in range(NST):
                        nc.tensor.matmul(
                            pv[0:DH + 1, it * 512:(it + 1) * 512],
                            lhsT=v_aug[jc][:, h * (DH + 1):(h + 1) * (DH + 1)],
                            rhs=ets[jc][:, it * 512:(it + 1) * 512],
                            start=(jc == 0),
                            stop=(jc == NSC - 1),
                        )

                trail = 1
                for jc in range(NSC):
                    et = epool.tile([128, S], bf16, name="et", tag="et")
                    ets[jc] = et
                    for half in range(2):
                        pa = psA.tile([128, 1024], f32, name="pa", tag="pa")
                        i0, i1 = 2 * half, 2 * half + 1
                        nc.tensor.matmul(
                            pa[:, 0:512],
                            lhsT=kt_lo[t][lo_sl, jc * 128:(jc + 1) * 128],
                            rhs=qt_lo[t][lo_sl, i0 * 512:(i0 + 1) * 512],
                        )
                        nc.tensor.matmul(
                            pa[:, 512:1024],
                            lhsT=kt_hi[t][hi_sl, jc * 128:(jc + 1) * 128],
                            rhs=qt_hi[t][hi_sl, i1 * 512:(i1 + 1) * 512],
                        )
                        nc.scalar.activation(
                            out=et[:, half * 1024:(half + 1) * 1024],
                            in_=pa[:, :],
                            func=Exp,
                            scale=SCALE,
                        )
                    if jc >= trail:
                        pv_mms(jc - trail)
                for jc in range(NSC - trail, NSC):
                    pv_mms(jc)

                # Decouple normalization from the PE pipeline: get O_un and
                # the denominator row out of PSUM fast, then normalize via
                # a cheap (128,16) reciprocal + DRAM partition-broadcast.
                oun = spool.tile([DH + 1, S], f32, name="oun", tag="oun")
                nc.vector.tensor_copy(oun[:, :], pv[0:DH + 1, :])
                den128 = spool.tile([128, 16], f32, name="den128", tag="d128")
                nc.sync.dma_start(out=den128[:, :], in_=oun[DH:DH + 1, :])
                nc.vector.reciprocal(out=den128[:, :], in_=den128[:, :])
                nc.sync.dma_start(out=den_dram[h, :], in_=den128[:, :])
                bc = spool.tile([64, S], f32, name="bc", tag="bc")
                dd = den_dram[h:h + 1, :]
                bcast_src = bass.AP(
                    tensor=dd.tensor,
                    offset=dd.offset,
                    ap=[[0, 64]] + [list(x) for x in dd.ap[1:]],
                )
                nc.sync.dma_start(out=bc[:, :], in_=bcast_src)
                nc.vector.tensor_mul(
                    ot[t][64 * p:64 * p + 64, :], oun[0:DH, :], bc[:, :])

            # ---- output projection (psA ping-pong so matmul groups and the
            # bias-add/copy of the previous group overlap) ----
            for ch in range(NDT):
                stage = opool.tile([128, S], f32, name="stage", tag="stage")
                for half in range(2):
                    po = psA.tile([128, 1024], f32, name="pa", tag="pa")
                    for st2 in range(2):
                        st = half * 2 + st2
                        for kt in range(NDT):
                            nc.tensor.matmul(
                                po[:, st2 * 512:(st2 + 1) * 512],
                                lhsT=wo_s[kt][:, ch * 128:(ch + 1) * 128],
                                rhs=ot[kt][:, st * 512:(st + 1) * 512],
                                start=(kt == 0),
                                stop=(kt == NDT - 1),
                            )
                    nc.vector.tensor_scalar_add(
                        out=stage[:, half * 1024:(half + 1) * 1024],
                        in0=po[:, :],
                        scalar1=bo_s[ch][:, :],
                    )
                    nc.sync.dma_start(
                        out=out[ch * 128:(ch + 1) * 128,
                                half * 1024:(half + 1) * 1024],
                        in_=stage[:, half * 1024:(half + 1) * 1024],
                    )

    nc.finalize()
    return nc


_NC_CACHE = None


def _get_nc():
    global _NC_CACHE
    if _NC_CACHE is None:
        _NC_CACHE = _build_kernel()
    return _NC_CACHE


def kernel(x, W_qkv, W_out, b_out):
    from concourse.bass_utils import run_bass_kernel_spmd

    bf16 = ml_dtypes.bfloat16

    # head-interleave and transpose the qkv weight: row 192h+{0,64,128}+c of
    # W_qkv is q/k/v row (h, c); regroup to e' = 64h+c and transpose to [d, e']
    w3 = W_qkv.reshape(H, 3, DH, D)
    wq_h = np.ascontiguousarray(w3[:, 0].reshape(INNER, D).T).astype(bf16)
    wk_h = np.ascontiguousarray(w3[:, 1].reshape(INNER, D).T).astype(bf16)
    wv_h = np.ascontiguousarray(w3[:, 2].reshape(INNER, D).T).astype(bf16)
    wo_h = np.ascontiguousarray(W_out.T).astype(bf16)  # [hc, d]
    bo_h = np.ascontiguousarray(b_out.reshape(NDT, 128, 1)).astype(np.float32)

    in_maps = []
    for b in range(N_CORES):
        xT_b = np.ascontiguousarray(x[b].T).astype(bf16)  # [d, s]
        in_maps.append({
            "xT": xT_b, "wq": wq_h, "wk": wk_h, "wv": wv_h,
            "wo": wo_h, "bo": bo_h,
        })

    nc = _get_nc()
    res = run_bass_kernel_spmd(nc, in_maps, list(range(N_CORES)))
    outs = [res.results[b]["out"].T for b in range(N_CORES)]  # [s, d] each
    return np.ascontiguousarray(np.stack(outs, axis=0)).astype(np.float32)

